# revision 32
# baseline (speedup 1.0000x reference)
"""Multi-head self-attention with RoPE on 8 Trainium2 NeuronCores.

Sharding: data-parallel over batch (2) x tensor-parallel over heads (4 groups
of 4 heads). Each core computes its heads' attention plus a partial output
projection (row-sharded Wo); the host sums the 4 partials per batch.

v2 design (bf16 matmul inputs, f32 PSUM):
  - All matmul operands bf16: halves input DMA, avoids f32r narrow-tile
    penalty, enables DVE 2x modes.
  - Projections run d-outer so PE starts as soon as the first x/w tiles land.
  - Scores transposed PT[k, q] per kb stripe; exp on ACT per whole suffix
    (1 op per (head, kb)); AV in NATURAL orientation out[q, 65] (65-col
    matmuls cost 65 cycles vs q-width in transposed form -> ~2x less PE).
  - V carries an all-ones column per head so AV also yields the softmax
    denominator per q IN the same partition as q -> normalization is one
    reciprocal + per-partition tensor_scalar multiply (no partition
    broadcast, no gpsimd).
  - attn [q, e] is PE-transposed per 128-block (identity matmul) for the
    natural-layout Wo matmul; Wo fused into head 3's per-qt completion.
  - Phase order [V][QKe0][h0][Qe1][h1][Ke1][h2][h3+out] interleaves the
    ACT-heavy exp of early heads under PE-only projection passes.
"""

import os
import sys

import numpy as np

for _p in ("/opt/trn_rl_repo", "/root/.axon_site/_ro/trn_rl_repo"):
    if os.path.isdir(_p) and _p not in sys.path:
        sys.path.insert(0, _p)
        break

import concourse.bacc as bacc
import concourse.tile as tile
from concourse import mybir
from concourse.bass_utils import run_bass_kernel_spmd

B, S, D, H = 2, 2048, 1024, 16
DK = 64
THETA = 10000.0
NCORES = 8
HPC = H // (NCORES // B)  # heads per core = 4
E = HPC * DK              # local dims per core = 256
DT8 = D // 128            # 8 d-tiles
ST = S // 128             # 16 s-tiles
CH = S // 512             # 4 512-chunks
F32 = mybir.dt.float32
BF = mybir.dt.bfloat16

_cache = {}


def _chunks(a, b):
    """Split [a, b) at absolute multiples of 512 (PSUM bank boundaries)."""
    out = []
    c0 = a
    while c0 < b:
        c1 = min(b, (c0 // 512 + 1) * 512)
        out.append((c0, c1))
        c0 = c1
    return out


def _build_nc():
    nc = bacc.Bacc(
        "TRN2",
        target_bir_lowering=False,
        debug=False,
        enable_asserts=False,
        num_devices=NCORES,
    )

    def mm(out, lhsT, rhs, **kw):
        nc.tensor.matmul(out, lhsT, rhs, **kw)

    # ---- I/O (all packed on host into SBUF-ready [128, *] layouts) ----
    x_d = nc.dram_tensor("x_d", [128, DT8 * S], BF, kind="ExternalInput").ap()
    wq_d = nc.dram_tensor("wq_d", [128, DT8 * E], BF, kind="ExternalInput").ap()
    wk_d = nc.dram_tensor("wk_d", [128, DT8 * E], BF, kind="ExternalInput").ap()
    wv_d = nc.dram_tensor("wv_d", [128, DT8 * E], BF, kind="ExternalInput").ap()
    wo_d = nc.dram_tensor("wo_d", [128, 2 * D], BF, kind="ExternalInput").ap()
    cos_d = nc.dram_tensor("cos_d", [128, S], BF, kind="ExternalInput").ap()
    sin_d = nc.dram_tensor("sin_d", [128, S], BF, kind="ExternalInput").ap()
    trid_d = nc.dram_tensor("trid_d", [128, 256], BF, kind="ExternalInput").ap()
    out_d = nc.dram_tensor("out", [S, D], F32, kind="ExternalOutput").ap()

    with tile.TileContext(nc) as tc:
      with (
          tc.tile_pool(name="const", bufs=1) as cp,
          tc.tile_pool(name="pers", bufs=1) as pp,
      ):
        cos = cp.tile([128, S], BF, name="cos", tag="cos")
        sin = cp.tile([128, S], BF, name="sin", tag="sin")
        trid = cp.tile([128, 256], BF, name="trid", tag="trid")
        wq_sb = pp.tile([128, DT8 * E], BF, name="wq", tag="wq")
        wk_sb = pp.tile([128, DT8 * E], BF, name="wk", tag="wk")
        wv_sb = pp.tile([128, DT8 * E], BF, name="wv", tag="wv")
        wo_sb = pp.tile([128, 2 * D], BF, name="wo", tag="wo")
        x_sb = [pp.tile([128, S], BF, name=f"x{d}", tag=f"x{d}") for d in range(DT8)]
        # per-512-chunk q/k tiles so scores only wait on the chunks they read
        qtc = [[pp.tile([128, 512], BF, name=f"qt{e}c{c}", tag=f"qt{e}c{c}")
                for c in range(CH)] for e in range(2)]
        ktc = [[pp.tile([128, 512], BF, name=f"kt{e}c{c}", tag=f"kt{e}c{c}")
                for c in range(CH)] for e in range(2)]
        v_all = pp.tile([128, ST * (E + HPC)], BF, name="vall", tag="vall")
        attn = pp.tile([128, ST * E], BF, name="attn", tag="attn")

        # ---- input loads: single queue => DMA engines serve in this order.
        # wv/x0 split in halves so the first V matmul starts sooner.
        nc.sync.dma_start(out=wv_sb[:, 0:512], in_=wv_d[:, 0:512])
        nc.scalar.dma_start(out=x_sb[0][:, 0:512], in_=x_d[:, 0:512])
        nc.sync.dma_start(out=wv_sb[:, 512:2048], in_=wv_d[:, 512:2048])
        nc.sync.dma_start(out=x_sb[0][:, 512:2048], in_=x_d[:, 512:2048])
        for d in range(1, DT8):
            nc.sync.dma_start(out=x_sb[d], in_=x_d[:, d * S:(d + 1) * S])
        nc.sync.dma_start(out=wq_sb, in_=wq_d)
        nc.scalar.dma_start(out=cos, in_=cos_d)
        nc.scalar.dma_start(out=sin, in_=sin_d)
        nc.sync.dma_start(out=wk_sb, in_=wk_d)
        nc.scalar.dma_start(out=trid, in_=trid_d)
        nc.scalar.dma_start(out=wo_sb, in_=wo_d)

        # ones columns for the softmax denominators
        nc.gpsimd.memset(
            v_all.rearrange("p (st h c) -> p st h c", st=ST, h=HPC)[:, :, :, 64:65],
            1.0)

        # ---- V projection pass (d-outer; all 8 PSUM banks) ----
        with tc.tile_pool(name="psV", bufs=1, space="PSUM") as psV:
            pv = [psV.tile([128, 512], F32, name=f"pv{i}", tag=f"pv{i}")
                  for i in range(8)]
            # one accumulation group per PSUM bank (2KB zero region): only the
            # bank's first chain sets start, only its last mm sets stop
            vv = v_all.rearrange("p (st h c) -> p st h c", st=ST, h=HPC)
            for d in range(DT8):
                for st in range(ST):
                    mm(pv[st // 2][:, (st % 2) * 256:(st % 2) * 256 + 256],
                       lhsT=x_sb[d][:, st * 128:(st + 1) * 128],
                       rhs=wv_sb[:, d * E:(d + 1) * E],
                       start=(d == 0 and st % 2 == 0),
                       stop=(d == DT8 - 1 and st % 2 == 1))
            for st in range(ST):
                srcv = pv[st // 2][:, (st % 2) * 256:(st % 2) * 256 + 256]
                srcv = srcv.rearrange("p (h c) -> p h c", h=HPC)
                if st % 2 == 0:
                    nc.scalar.copy(out=vv[:, st, :, 0:64], in_=srcv)
                else:
                    nc.vector.tensor_copy(out=vv[:, st, :, 0:64], in_=srcv)

        # ---- Q/K projection: chunk-serial steps (1-2 PSUM banks each);
        # returned as closures so heads can interleave them into exp gaps ----
        def proj_steps(w_sb, dstc, e, psF, rpF, bufs, ptag="psF"):
            tsin = rpF.tile([128, S], BF, name="tsin", tag="tsin")
            tsw = rpF.tile([128, S], BF, name="tsw", tag="tsw")

            def step(c):
                ps = psF.tile([128, 512], F32, name="psF", tag=ptag, bufs=bufs)
                for d in range(DT8):
                    mm(ps,
                       lhsT=w_sb[:, d * E + e * 128:d * E + e * 128 + 128],
                       rhs=x_sb[d][:, c * 512:(c + 1) * 512],
                       start=(d == 0), stop=(d == DT8 - 1))
                sl = slice(c * 512, (c + 1) * 512)
                nc.vector.tensor_mul(out=tsin[:, sl], in0=ps, in1=sin[:, sl])
                nc.vector.tensor_mul(out=dstc[c][:, :], in0=ps, in1=cos[:, sl])
                if c % 2 == 1:
                    # rotate-half swap for the finished chunk pair
                    psl = slice((c - 1) * 512, (c + 1) * 512)
                    for b in range(4):
                        nc.sync.dma_start(
                            out=tsw[b * 32:(b + 1) * 32, psl],
                            in_=tsin[(b ^ 1) * 32:((b ^ 1) + 1) * 32, psl])
                    for cc in (c - 1, c):
                        ssl = slice(cc * 512, (cc + 1) * 512)
                        nc.gpsimd.tensor_add(out=dstc[cc][:, :],
                                             in0=dstc[cc][:, :],
                                             in1=tsw[:, ssl])

            return [lambda c=c: step(c) for c in range(CH)]

        # ---- attention head; seg-level software pipeline ----
        # Per kb the causal suffix [q0, 2048) is processed in 1024-col
        # segments A=[q0,1024) (kb<8 only) and B=[1024,2048) / [q0,2048).
        # exp(A) and exp(B) go to separate pte tiles so AV mms only wait on
        # the half they read; AV for the B-half of kb-1 is emitted after
        # kb's scores, keeping PE busy while ACT runs kb's exp.
        def head(h, filler=(), slots=None, fuse_out=False):
            e, hb = h // 2, (h % 2) * 64
            filler = list(filler)
            if slots is None:
                slots = {3: 0, 6: 1, 9: 2, 12: 3} if len(filler) == 4 else \
                        {4: 0, 9: 1} if len(filler) == 2 else {}
            with (
                tc.tile_pool(name="psH", bufs=1, space="PSUM") as psH,
                tc.tile_pool(name="sbH", bufs=1) as sbH,
            ):
                avq = [psH.tile([128, n * 65], F32, name=f"av{t}", tag=f"av{t}")
                       for t, n in ((0, 7), (1, 7), (2, 2))]

                def tail(qt_i):
                    # normalize + transpose + Wo + store for one q-tile
                    norm(qt_i)
                    atnT = sbH.tile([128, 256], BF, name="atnT", tag="atnT",
                                    bufs=2)
                    for half in range(2):
                        tp = psH.tile([128, 128], BF, name="tp", tag="tp",
                                      bufs=1)
                        nc.tensor.transpose(
                            tp, attn[:, qt_i * E + half * 128:qt_i * E + half * 128 + 128],
                            trid[:, 128:256])
                        nc.vector.tensor_copy(
                            out=atnT[:, half * 128:half * 128 + 128], in_=tp)
                    for oc in range(2):
                        po = psH.tile([128, 512], F32, name=f"po{oc}",
                                      tag=f"po{oc}", bufs=1)
                        mm(po, lhsT=atnT[:, 0:128],
                           rhs=wo_sb[:, oc * 512:oc * 512 + 512],
                           start=True, stop=False)
                        mm(po, lhsT=atnT[:, 128:256],
                           rhs=wo_sb[:, D + oc * 512:D + oc * 512 + 512],
                           start=False, stop=True)
                        po_sb = sbH.tile([128, 512], F32, name="posb",
                                         tag="posb", bufs=4)
                        if (qt_i * 2 + oc) % 2 == 0:
                            nc.scalar.copy(out=po_sb, in_=po)
                        else:
                            nc.vector.tensor_copy(out=po_sb, in_=po)
                        q_eng = nc.scalar if (qt_i >= 14 and oc == 1) else nc.sync
                        q_eng.dma_start(
                            out=out_d[qt_i * 128:(qt_i + 1) * 128,
                                      oc * 512:(oc + 1) * 512],
                            in_=po_sb)

                def norm(qt_i):
                    t, j = qt_i // 7, qt_i % 7
                    rec = sbH.tile([128, 1], F32, name="rec", tag="rec", bufs=2)
                    nc.vector.reciprocal(
                        out=rec, in_=avq[t][:, j * 65 + 64:j * 65 + 65])
                    nc.vector.tensor_scalar_mul(
                        out=attn[:, qt_i * E + h * 64:qt_i * E + h * 64 + 64],
                        in0=avq[t][:, j * 65:j * 65 + 64], scalar1=rec[:, 0:1])

                def av_mm(kb, qi, pte, base):
                    t, j = qi // 7, qi % 7
                    last_qt = 15 if t == 2 else t * 7 + 6
                    mm(avq[t][:, j * 65:j * 65 + 65],
                       lhsT=pte[:, qi * 128 - base:qi * 128 - base + 128],
                       rhs=v_all[:, kb * 260 + h * 65:kb * 260 + h * 65 + 65],
                       start=(kb == 0 and j == 0),
                       stop=(qi == kb == last_qt))
                    if qi == kb:
                        if fuse_out:
                            tail(kb)
                        else:
                            norm(kb)

                pendB = []  # (kb, pteB, baseB) awaiting AV, drained 2-deep
                pendA = []  # (kb, pteA, baseA) awaiting AV, drained 1-deep
                stripe_bufs = 1 if fuse_out else 2
                for kb in range(ST):
                    q0 = kb * 128
                    cb = kb // 4
                    krow = ktc[e][cb][hb:hb + 64, q0 - cb * 512:q0 - cb * 512 + 128]
                    segs = ([(q0, 1024, "A"), (1024, S, "B")] if kb < 8
                            else [(q0, S, "B")])
                    cur = {}
                    for (sa, sb_, nm) in segs:
                        base = (sa // 512) * 512
                        pte = sbH.tile([128, 1024], BF, name=f"pte{nm}",
                                       tag=f"pte{nm}", bufs=3 if nm == "A" else 4)
                        stripe = psH.tile([128, 1024], F32, name="stripe",
                                          tag="stripe", bufs=stripe_bufs)
                        for (a, b) in _chunks(sa, sb_):
                            c = a // 512
                            mm(stripe[:, a - base:b - base],
                               lhsT=krow,
                               rhs=qtc[e][c][hb:hb + 64, a - c * 512:b - c * 512],
                               start=True, stop=True)
                        nc.scalar.activation(
                            out=pte[:, sa - base:sb_ - base],
                            in_=stripe[:, sa - base:sb_ - base],
                            func=mybir.ActivationFunctionType.Exp, scale=0.125)
                        cur[nm] = (pte, base)
                    dm, (dpte, dbase) = ("A", cur["A"]) if kb < 8 else ("B", cur["B"])
                    nc.gpsimd.tensor_mul(out=dpte[:, q0 - dbase:q0 - dbase + 128],
                                         in0=dpte[:, q0 - dbase:q0 - dbase + 128],
                                         in1=trid[:, 0:128])
                    # PE-ready work while ACT runs this kb's exp: drain AV for
                    # segments whose exp finished 1-2 kbs ago
                    if pendA:
                        pkb, ppte, pbase = pendA.pop(0)
                        for qi in range(pkb, 8):
                            av_mm(pkb, qi, ppte, pbase)
                    if kb in slots:
                        filler[slots[kb]]()
                    if len(pendB) >= (1 if fuse_out else 2):
                        pkb, ppte, pbase = pendB.pop(0)
                        for qi in range(max(8, pkb), ST):
                            av_mm(pkb, qi, ppte, pbase)
                    if kb < 8:
                        pendA.append((kb, cur["A"][0], cur["A"][1]))
                    pendB.append((kb, cur["B"][0], cur["B"][1]))
                for pkb, ppte, pbase in pendA:
                    for qi in range(pkb, 8):
                        av_mm(pkb, qi, ppte, pbase)
                for pkb, ppte, pbase in pendB:
                    for qi in range(max(8, pkb), ST):
                        av_mm(pkb, qi, ppte, pbase)



        # ---- phase schedule ----
        # QK e0 chunk-serial up front (2-bank pipelining)
        with (
            tc.tile_pool(name="psF0", bufs=1, space="PSUM") as psF0,
            tc.tile_pool(name="rp0", bufs=1) as rp0,
        ):
            for s in proj_steps(wq_sb, qtc[0], 0, psF0, rp0, bufs=3, ptag="psFq"):
                s()
            for s in proj_steps(wk_sb, ktc[0], 0, psF0, rp0, bufs=2, ptag="psFk"):
                s()
        # h0 with Qe1 interleaved, h1/h2 with Ke1 halves interleaved
        with (
            tc.tile_pool(name="psF1", bufs=1, space="PSUM") as psF1,
            tc.tile_pool(name="rp1", bufs=1) as rp1,
        ):
            head(0, filler=proj_steps(wq_sb, qtc[1], 1, psF1, rp1, bufs=1))
        with (
            tc.tile_pool(name="psF2", bufs=1, space="PSUM") as psF2,
            tc.tile_pool(name="rp2", bufs=1) as rp2,
        ):
            ke1 = proj_steps(wk_sb, ktc[1], 1, psF2, rp2, bufs=1)
            head(1, filler=ke1[:2])
            # c3's step carries the pair's rotate-half adds; both chunks must
            # land before h2's kb=8 scores read ktc[1][2]
            head(2, filler=ke1[2:], slots={2: 0, 4: 1})
        head(3, fuse_out=True)

    nc.compile()
    return nc


def _host_inputs(x, token_positions, Wq, Wk, Wv, Wo):
    import ml_dtypes
    bf = ml_dtypes.bfloat16

    def pack(a, nblk):
        # [nblk*128, w] -> [128, nblk*w] with block d at cols [d*w, (d+1)*w)
        w = a.shape[1]
        return np.ascontiguousarray(
            a.reshape(nblk, 128, w).transpose(1, 0, 2).reshape(128, nblk * w))

    perm = np.concatenate([np.arange(0, DK, 2), np.arange(1, DK, 2)])
    inv_freq = THETA ** (-np.arange(0, DK, 2, dtype=np.float64) / DK)
    trid = np.concatenate(
        [np.triu(np.ones((128, 128), np.float32)), np.eye(128, dtype=np.float32)],
        axis=1)
    in_maps = []
    for c in range(NCORES):
        b, g = divmod(c, NCORES // B)
        heads = [(g * HPC + h) for h in range(HPC)]
        rows_rope = np.concatenate([h * DK + perm for h in heads])
        rows_plain = np.concatenate([h * DK + np.arange(DK) for h in heads])
        pos = token_positions[b].astype(np.float64)
        ang = inv_freq[:, None] * pos[None, :]  # [32, S]
        cosv = np.cos(ang).astype(np.float32)
        sinv = np.sin(ang).astype(np.float32)
        cosF = np.concatenate([cosv] * 4, axis=0)  # [128, S]
        sinF = np.concatenate([sinv, -sinv, sinv, -sinv], axis=0)
        in_maps.append({
            "x_d": pack(np.ascontiguousarray(x[b].T), DT8).astype(bf),
            "wq_d": pack(np.ascontiguousarray(Wq[rows_rope, :].T), DT8).astype(bf),
            "wk_d": pack(np.ascontiguousarray(Wk[rows_rope, :].T), DT8).astype(bf),
            "wv_d": pack(np.ascontiguousarray(Wv[rows_plain, :].T), DT8).astype(bf),
            "wo_d": pack(np.ascontiguousarray(Wo[:, rows_plain].T), 2).astype(bf),
            "cos_d": cosF.astype(bf),
            "sin_d": sinF.astype(bf),
            "trid_d": trid.astype(bf),
        })
    return in_maps


def kernel(x, token_positions, Wq, Wk, Wv, Wo, _debug=False):
    x = np.asarray(x, np.float32)
    token_positions = np.asarray(token_positions, np.int32)
    Wq, Wk, Wv, Wo = (np.asarray(w, np.float32) for w in (Wq, Wk, Wv, Wo))
    if "nc" not in _cache:
        _cache["nc"] = _build_nc()
    nc = _cache["nc"]
    in_maps = _host_inputs(x, token_positions, Wq, Wk, Wv, Wo)
    res = run_bass_kernel_spmd(
        nc, in_maps, core_ids=list(range(NCORES)), trace=False)
    outs = [r["out"] for r in res.results]
    full = np.zeros((B, S, D), np.float32)
    for c in range(NCORES):
        full[c // (NCORES // B)] += outs[c]
    if _debug:
        return full, res
    return full


# revision 33
# speedup vs baseline: 1.0311x; 1.0311x over previous
"""Multi-head self-attention with RoPE on 8 Trainium2 NeuronCores.

Sharding: data-parallel over batch (2) x tensor-parallel over heads (4 groups
of 4 heads). Each core computes its heads' attention plus a partial output
projection (row-sharded Wo); the host sums the 4 partials per batch.

v2 design (bf16 matmul inputs, f32 PSUM):
  - All matmul operands bf16: halves input DMA, avoids f32r narrow-tile
    penalty, enables DVE 2x modes.
  - Projections run d-outer so PE starts as soon as the first x/w tiles land.
  - Scores transposed PT[k, q] per kb stripe; exp on ACT per whole suffix
    (1 op per (head, kb)); AV in NATURAL orientation out[q, 65] (65-col
    matmuls cost 65 cycles vs q-width in transposed form -> ~2x less PE).
  - V carries an all-ones column per head so AV also yields the softmax
    denominator per q IN the same partition as q -> normalization is one
    reciprocal + per-partition tensor_scalar multiply (no partition
    broadcast, no gpsimd).
  - attn [q, e] is PE-transposed per 128-block (identity matmul) for the
    natural-layout Wo matmul; Wo fused into head 3's per-qt completion.
  - Phase order [V][QKe0][h0][Qe1][h1][Ke1][h2][h3+out] interleaves the
    ACT-heavy exp of early heads under PE-only projection passes.
"""

import os
import sys

import numpy as np

for _p in ("/opt/trn_rl_repo", "/root/.axon_site/_ro/trn_rl_repo"):
    if os.path.isdir(_p) and _p not in sys.path:
        sys.path.insert(0, _p)
        break

import concourse.bacc as bacc
import concourse.tile as tile
from concourse import mybir
from concourse.bass_utils import run_bass_kernel_spmd

B, S, D, H = 2, 2048, 1024, 16
DK = 64
THETA = 10000.0
NCORES = 8
HPC = H // (NCORES // B)  # heads per core = 4
E = HPC * DK              # local dims per core = 256
DT8 = D // 128            # 8 d-tiles
ST = S // 128             # 16 s-tiles
CH = S // 512             # 4 512-chunks
F32 = mybir.dt.float32
BF = mybir.dt.bfloat16

_cache = {}


def _chunks(a, b):
    """Split [a, b) at absolute multiples of 512 (PSUM bank boundaries)."""
    out = []
    c0 = a
    while c0 < b:
        c1 = min(b, (c0 // 512 + 1) * 512)
        out.append((c0, c1))
        c0 = c1
    return out


def _build_nc():
    nc = bacc.Bacc(
        "TRN2",
        target_bir_lowering=False,
        debug=False,
        enable_asserts=False,
        num_devices=NCORES,
    )

    def mm(out, lhsT, rhs, **kw):
        nc.tensor.matmul(out, lhsT, rhs, **kw)

    # ---- I/O (all packed on host into SBUF-ready [128, *] layouts) ----
    x_d = nc.dram_tensor("x_d", [128, DT8 * S], BF, kind="ExternalInput").ap()
    wq_d = nc.dram_tensor("wq_d", [128, DT8 * E], BF, kind="ExternalInput").ap()
    wk_d = nc.dram_tensor("wk_d", [128, DT8 * E], BF, kind="ExternalInput").ap()
    wv_d = nc.dram_tensor("wv_d", [128, DT8 * E], BF, kind="ExternalInput").ap()
    wo_d = nc.dram_tensor("wo_d", [128, 2 * D], BF, kind="ExternalInput").ap()
    cos_d = nc.dram_tensor("cos_d", [128, S], BF, kind="ExternalInput").ap()
    sin_d = nc.dram_tensor("sin_d", [128, S], BF, kind="ExternalInput").ap()
    trid_d = nc.dram_tensor("trid_d", [128, 256], BF, kind="ExternalInput").ap()
    out_d = nc.dram_tensor("out", [S, D], F32, kind="ExternalOutput").ap()

    with tile.TileContext(nc) as tc:
      with (
          tc.tile_pool(name="const", bufs=1) as cp,
          tc.tile_pool(name="pers", bufs=1) as pp,
      ):
        cos = cp.tile([128, S], BF, name="cos", tag="cos")
        sin = cp.tile([128, S], BF, name="sin", tag="sin")
        trid = cp.tile([128, 256], BF, name="trid", tag="trid")
        wq_sb = pp.tile([128, DT8 * E], BF, name="wq", tag="wq")
        wk_sb = pp.tile([128, DT8 * E], BF, name="wk", tag="wk")
        wv_sb = pp.tile([128, DT8 * E], BF, name="wv", tag="wv")
        wo_sb = pp.tile([128, 2 * D], BF, name="wo", tag="wo")
        x_sb = [pp.tile([128, S], BF, name=f"x{d}", tag=f"x{d}") for d in range(DT8)]
        # per-512-chunk q/k tiles so scores only wait on the chunks they read
        qtc = [[pp.tile([128, 512], BF, name=f"qt{e}c{c}", tag=f"qt{e}c{c}")
                for c in range(CH)] for e in range(2)]
        ktc = [[pp.tile([128, 512], BF, name=f"kt{e}c{c}", tag=f"kt{e}c{c}")
                for c in range(CH)] for e in range(2)]
        v_all = pp.tile([128, ST * (E + HPC)], BF, name="vall", tag="vall")
        attn = pp.tile([128, ST * E], BF, name="attn", tag="attn")

        # ---- input loads: single queue => DMA engines serve in this order.
        # wv/x0 split in halves so the first V matmul starts sooner.
        nc.sync.dma_start(out=wv_sb[:, 0:512], in_=wv_d[:, 0:512])
        nc.scalar.dma_start(out=x_sb[0][:, 0:512], in_=x_d[:, 0:512])
        nc.sync.dma_start(out=wv_sb[:, 512:2048], in_=wv_d[:, 512:2048])
        nc.sync.dma_start(out=x_sb[0][:, 512:2048], in_=x_d[:, 512:2048])
        for d in range(1, DT8):
            nc.sync.dma_start(out=x_sb[d], in_=x_d[:, d * S:(d + 1) * S])
        nc.sync.dma_start(out=wq_sb, in_=wq_d)
        nc.scalar.dma_start(out=cos, in_=cos_d)
        nc.scalar.dma_start(out=sin, in_=sin_d)
        nc.sync.dma_start(out=wk_sb, in_=wk_d)
        nc.scalar.dma_start(out=trid, in_=trid_d)
        nc.scalar.dma_start(out=wo_sb, in_=wo_d)

        # ones columns for the softmax denominators
        nc.gpsimd.memset(
            v_all.rearrange("p (st h c) -> p st h c", st=ST, h=HPC)[:, :, :, 64:65],
            1.0)

        # ---- V projection pass (d-outer; all 8 PSUM banks) ----
        with tc.tile_pool(name="psV", bufs=1, space="PSUM") as psV:
            pv = [psV.tile([128, 512], F32, name=f"pv{i}", tag=f"pv{i}")
                  for i in range(8)]
            # one accumulation group per PSUM bank (2KB zero region): only the
            # bank's first chain sets start, only its last mm sets stop
            vv = v_all.rearrange("p (st h c) -> p st h c", st=ST, h=HPC)
            for d in range(DT8):
                for st in range(ST):
                    mm(pv[st // 2][:, (st % 2) * 256:(st % 2) * 256 + 256],
                       lhsT=x_sb[d][:, st * 128:(st + 1) * 128],
                       rhs=wv_sb[:, d * E:(d + 1) * E],
                       start=(d == 0 and st % 2 == 0),
                       stop=(d == DT8 - 1 and st % 2 == 1))
            for st in range(ST):
                srcv = pv[st // 2][:, (st % 2) * 256:(st % 2) * 256 + 256]
                srcv = srcv.rearrange("p (h c) -> p h c", h=HPC)
                if st % 2 == 0:
                    nc.scalar.copy(out=vv[:, st, :, 0:64], in_=srcv)
                else:
                    nc.vector.tensor_copy(out=vv[:, st, :, 0:64], in_=srcv)

        # ---- Q/K projection: chunk-serial steps (1-2 PSUM banks each);
        # returned as closures so heads can interleave them into exp gaps ----
        def proj_steps(w_sb, dstc, e, psF, rpF, bufs, ptag="psF"):
            tsin = rpF.tile([128, S], BF, name="tsin", tag="tsin")
            tsw = rpF.tile([128, S], BF, name="tsw", tag="tsw")

            def step(c):
                ps = psF.tile([128, 512], F32, name="psF", tag=ptag, bufs=bufs)
                for d in range(DT8):
                    mm(ps,
                       lhsT=w_sb[:, d * E + e * 128:d * E + e * 128 + 128],
                       rhs=x_sb[d][:, c * 512:(c + 1) * 512],
                       start=(d == 0), stop=(d == DT8 - 1))
                sl = slice(c * 512, (c + 1) * 512)
                nc.vector.tensor_mul(out=tsin[:, sl], in0=ps, in1=sin[:, sl])
                nc.vector.tensor_mul(out=dstc[c][:, :], in0=ps, in1=cos[:, sl])
                if c % 2 == 1:
                    # rotate-half swap for the finished chunk pair
                    psl = slice((c - 1) * 512, (c + 1) * 512)
                    for b in range(4):
                        nc.sync.dma_start(
                            out=tsw[b * 32:(b + 1) * 32, psl],
                            in_=tsin[(b ^ 1) * 32:((b ^ 1) + 1) * 32, psl])
                    for cc in (c - 1, c):
                        ssl = slice(cc * 512, (cc + 1) * 512)
                        nc.gpsimd.tensor_add(out=dstc[cc][:, :],
                                             in0=dstc[cc][:, :],
                                             in1=tsw[:, ssl])

            return [lambda c=c: step(c) for c in range(CH)]

        # ---- attention head; seg-level software pipeline ----
        # Per kb the causal suffix [q0, 2048) is processed in 1024-col
        # segments A=[q0,1024) (kb<8 only) and B=[1024,2048) / [q0,2048).
        # exp(A) and exp(B) go to separate pte tiles so AV mms only wait on
        # the half they read; AV for the B-half of kb-1 is emitted after
        # kb's scores, keeping PE busy while ACT runs kb's exp.
        def head(h, filler=(), slots=None, fuse_out=False):
            e, hb = h // 2, (h % 2) * 64
            filler = list(filler)
            if slots is None:
                slots = {3: 0, 6: 1, 9: 2, 12: 3} if len(filler) == 4 else \
                        {4: 0, 9: 1} if len(filler) == 2 else {}
            with (
                tc.tile_pool(name="psH", bufs=1, space="PSUM") as psH,
                tc.tile_pool(name="sbH", bufs=1) as sbH,
            ):
                avq = [psH.tile([128, n * 65], F32, name=f"av{t}", tag=f"av{t}")
                       for t, n in ((0, 7), (1, 7), (2, 2))]

                def tail(qt_i):
                    # normalize + transpose + Wo + store for one q-tile
                    norm(qt_i)
                    atnT = sbH.tile([128, 256], BF, name="atnT", tag="atnT",
                                    bufs=2)
                    for half in range(2):
                        tp = psH.tile([128, 128], BF, name="tp", tag="tp",
                                      bufs=1)
                        nc.tensor.transpose(
                            tp, attn[:, qt_i * E + half * 128:qt_i * E + half * 128 + 128],
                            trid[:, 128:256])
                        nc.vector.tensor_copy(
                            out=atnT[:, half * 128:half * 128 + 128], in_=tp)
                    for oc in range(2):
                        po = psH.tile([128, 512], F32, name=f"po{oc}",
                                      tag=f"po{oc}", bufs=1)
                        mm(po, lhsT=atnT[:, 0:128],
                           rhs=wo_sb[:, oc * 512:oc * 512 + 512],
                           start=True, stop=False)
                        mm(po, lhsT=atnT[:, 128:256],
                           rhs=wo_sb[:, D + oc * 512:D + oc * 512 + 512],
                           start=False, stop=True)
                        po_sb = sbH.tile([128, 512], F32, name="posb",
                                         tag="posb", bufs=4)
                        if (qt_i * 2 + oc) % 2 == 0:
                            nc.scalar.copy(out=po_sb, in_=po)
                        else:
                            nc.vector.tensor_copy(out=po_sb, in_=po)
                        q_eng = nc.scalar if (qt_i >= 14 and oc == 1) else nc.sync
                        q_eng.dma_start(
                            out=out_d[qt_i * 128:(qt_i + 1) * 128,
                                      oc * 512:(oc + 1) * 512],
                            in_=po_sb)

                def norm(qt_i):
                    t, j = qt_i // 7, qt_i % 7
                    rec = sbH.tile([128, 1], F32, name="rec", tag="rec", bufs=2)
                    nc.vector.reciprocal(
                        out=rec, in_=avq[t][:, j * 65 + 64:j * 65 + 65])
                    nc.vector.tensor_scalar_mul(
                        out=attn[:, qt_i * E + h * 64:qt_i * E + h * 64 + 64],
                        in0=avq[t][:, j * 65:j * 65 + 64], scalar1=rec[:, 0:1])

                def av_mm(kb, qi, pte, base):
                    t, j = qi // 7, qi % 7
                    last_qt = 15 if t == 2 else t * 7 + 6
                    mm(avq[t][:, j * 65:j * 65 + 65],
                       lhsT=pte[:, qi * 128 - base:qi * 128 - base + 128],
                       rhs=v_all[:, kb * 260 + h * 65:kb * 260 + h * 65 + 65],
                       start=(kb == 0 and j == 0),
                       stop=(qi == kb == last_qt))
                    if qi == kb:
                        if fuse_out:
                            tail(kb)
                        else:
                            norm(kb)

                pendB = []  # (kb, pteB, baseB) awaiting AV, drained 2-deep
                pendA = []  # (kb, pteA, baseA) awaiting AV, drained 1-deep
                stripe_bufs = 1 if fuse_out else 2
                for kb in range(ST):
                    q0 = kb * 128
                    cb = kb // 4
                    krow = ktc[e][cb][hb:hb + 64, q0 - cb * 512:q0 - cb * 512 + 128]
                    segs = ([(q0, 1024, "A"), (1024, S, "B")] if kb < 8
                            else [(q0, S, "B")])
                    cur = {}
                    for (sa, sb_, nm) in segs:
                        base = (sa // 512) * 512
                        pte = sbH.tile([128, 1024], BF, name=f"pte{nm}",
                                       tag=f"pte{nm}", bufs=3 if nm == "A" else 4)
                        stripe = psH.tile([128, 1024], F32, name="stripe",
                                          tag="stripe", bufs=stripe_bufs)
                        for (a, b) in _chunks(sa, sb_):
                            c = a // 512
                            mm(stripe[:, a - base:b - base],
                               lhsT=krow,
                               rhs=qtc[e][c][hb:hb + 64, a - c * 512:b - c * 512],
                               start=True, stop=True)
                        nc.scalar.activation(
                            out=pte[:, sa - base:sb_ - base],
                            in_=stripe[:, sa - base:sb_ - base],
                            func=mybir.ActivationFunctionType.Exp, scale=0.125)
                        cur[nm] = (pte, base)
                    dm, (dpte, dbase) = ("A", cur["A"]) if kb < 8 else ("B", cur["B"])
                    nc.gpsimd.tensor_mul(out=dpte[:, q0 - dbase:q0 - dbase + 128],
                                         in0=dpte[:, q0 - dbase:q0 - dbase + 128],
                                         in1=trid[:, 0:128])
                    # PE-ready work while ACT runs this kb's exp: drain AV for
                    # segments whose exp finished 1-2 kbs ago
                    if pendA:
                        pkb, ppte, pbase = pendA.pop(0)
                        for qi in range(pkb, 8):
                            av_mm(pkb, qi, ppte, pbase)
                    if kb in slots:
                        filler[slots[kb]]()
                    if len(pendB) >= (1 if fuse_out else 2):
                        pkb, ppte, pbase = pendB.pop(0)
                        for qi in range(max(8, pkb), ST):
                            av_mm(pkb, qi, ppte, pbase)
                    if kb < 8:
                        pendA.append((kb, cur["A"][0], cur["A"][1]))
                    pendB.append((kb, cur["B"][0], cur["B"][1]))
                for pkb, ppte, pbase in pendA:
                    for qi in range(pkb, 8):
                        av_mm(pkb, qi, ppte, pbase)
                for pkb, ppte, pbase in pendB:
                    for qi in range(max(8, pkb), ST):
                        av_mm(pkb, qi, ppte, pbase)



        # ---- phase schedule ----
        # QK e0 chunk-serial up front (2-bank pipelining)
        with (
            tc.tile_pool(name="psF0", bufs=1, space="PSUM") as psF0,
            tc.tile_pool(name="rp0", bufs=1) as rp0,
        ):
            for s in proj_steps(wq_sb, qtc[0], 0, psF0, rp0, bufs=4):
                s()
            for s in proj_steps(wk_sb, ktc[0], 0, psF0, rp0, bufs=4):
                s()
        # h0 with Qe1 interleaved, h1/h2 with Ke1 halves interleaved
        with (
            tc.tile_pool(name="psF1", bufs=1, space="PSUM") as psF1,
            tc.tile_pool(name="rp1", bufs=1) as rp1,
        ):
            head(0, filler=proj_steps(wq_sb, qtc[1], 1, psF1, rp1, bufs=1))
        with (
            tc.tile_pool(name="psF2", bufs=1, space="PSUM") as psF2,
            tc.tile_pool(name="rp2", bufs=1) as rp2,
        ):
            ke1 = proj_steps(wk_sb, ktc[1], 1, psF2, rp2, bufs=1)
            head(1, filler=ke1[:2])
            # c3's step carries the pair's rotate-half adds; both chunks must
            # land before h2's kb=8 scores read ktc[1][2]
            head(2, filler=ke1[2:], slots={2: 0, 4: 1})
        head(3, fuse_out=True)

    nc.compile()
    return nc


def _host_inputs(x, token_positions, Wq, Wk, Wv, Wo):
    import ml_dtypes
    bf = ml_dtypes.bfloat16

    def pack(a, nblk):
        # [nblk*128, w] -> [128, nblk*w] with block d at cols [d*w, (d+1)*w)
        w = a.shape[1]
        return np.ascontiguousarray(
            a.reshape(nblk, 128, w).transpose(1, 0, 2).reshape(128, nblk * w))

    perm = np.concatenate([np.arange(0, DK, 2), np.arange(1, DK, 2)])
    inv_freq = THETA ** (-np.arange(0, DK, 2, dtype=np.float64) / DK)
    trid = np.concatenate(
        [np.triu(np.ones((128, 128), np.float32)), np.eye(128, dtype=np.float32)],
        axis=1)
    in_maps = []
    for c in range(NCORES):
        b, g = divmod(c, NCORES // B)
        heads = [(g * HPC + h) for h in range(HPC)]
        rows_rope = np.concatenate([h * DK + perm for h in heads])
        rows_plain = np.concatenate([h * DK + np.arange(DK) for h in heads])
        pos = token_positions[b].astype(np.float64)
        ang = inv_freq[:, None] * pos[None, :]  # [32, S]
        cosv = np.cos(ang).astype(np.float32)
        sinv = np.sin(ang).astype(np.float32)
        cosF = np.concatenate([cosv] * 4, axis=0)  # [128, S]
        sinF = np.concatenate([sinv, -sinv, sinv, -sinv], axis=0)
        in_maps.append({
            "x_d": pack(np.ascontiguousarray(x[b].T), DT8).astype(bf),
            "wq_d": pack(np.ascontiguousarray(Wq[rows_rope, :].T), DT8).astype(bf),
            "wk_d": pack(np.ascontiguousarray(Wk[rows_rope, :].T), DT8).astype(bf),
            "wv_d": pack(np.ascontiguousarray(Wv[rows_plain, :].T), DT8).astype(bf),
            "wo_d": pack(np.ascontiguousarray(Wo[:, rows_plain].T), 2).astype(bf),
            "cos_d": cosF.astype(bf),
            "sin_d": sinF.astype(bf),
            "trid_d": trid.astype(bf),
        })
    return in_maps


def kernel(x, token_positions, Wq, Wk, Wv, Wo, _debug=False):
    x = np.asarray(x, np.float32)
    token_positions = np.asarray(token_positions, np.int32)
    Wq, Wk, Wv, Wo = (np.asarray(w, np.float32) for w in (Wq, Wk, Wv, Wo))
    if "nc" not in _cache:
        _cache["nc"] = _build_nc()
    nc = _cache["nc"]
    in_maps = _host_inputs(x, token_positions, Wq, Wk, Wv, Wo)
    res = run_bass_kernel_spmd(
        nc, in_maps, core_ids=list(range(NCORES)), trace=False)
    outs = [r["out"] for r in res.results]
    full = np.zeros((B, S, D), np.float32)
    for c in range(NCORES):
        full[c // (NCORES // B)] += outs[c]
    if _debug:
        return full, res
    return full
